# revision 17
# baseline (speedup 1.0000x reference)
"""Trainium2 Bass kernel for nn_AttentionWithCache (decode attention with KV cache).

Full-input contract: kernel(**inputs) takes the unsharded numpy inputs and
returns the full [1, 128, 4096] output. Internally shards tensor-parallel
over heads across 8 NeuronCores (4 heads each), runs a Bass/Tile kernel via
run_bass_kernel_spmd, and reduces the wo partial sums on gather.

Key algebraic simplification: RoPE at a single scalar position applied to
both q and the whole live cache cancels in q.k dot products; v is never
rotated — so the kernel skips RoPE entirely.

Softmax subtracts a constant 13 inside exp() (cancels in the ratio); the
denominator comes from a ones-column appended to v.

v2 redesign (from trace analysis of the 83-85us baseline):
  - EVERY streamed tile has a dedicated SBUF buffer. The baseline aliased
    stream pools (v2 reused v0's buffer etc), which made later dma_starts
    wait on compute semaphores and head-of-line-blocked the single sync
    DMA queue: k1 landed at 56us and woB at 72us, gating the whole tail.
  - Single consumption-ordered gapless stream: x, wq0, wq1, wk0, wv0, k0,
    wk1, wk2, wv1, wk3, k1, v0, v1, wq2, wq3, v2, v3, wo0..wo3. k1 lands
    ~36us so the ACT-bound exp pipeline (4 heads x 4.6us) runs with only
    one gap; wo (f8, 4 pieces) lands last feeding the output-projection
    tail directly.
  - Biases folded into the projection matmuls via a ones-row extra
    contraction chunk (kills the 0.79MB broadcast DMAs).
  - wo stored f8 e3m4 x128 (saves 2MB off the end of the stream).
  - Finales (new-row scores + normalization) run after all attention
    heads: pN exps for h0/h1 slot into the ACT gap between h1 and h2.
  - PSUM plan (8 banks, bank-granular): EARLY(qps,tr,kvk,kvv)=4 + KQ(2x2)
    =4; EARLY released after kv-proj evict, LATE(av x2, fin x2)=4; KQ
    released after last exp, Y(2x2)=4.
  - y output DMAs ride the gpsimd queue so they don't queue behind the
    input stream on sync.
"""

import sys

if "/opt/trn_rl_repo" not in sys.path:
    sys.path.insert(0, "/opt/trn_rl_repo")

import ml_dtypes
import numpy as np

import concourse.bass as bass
import concourse.mybir as mybir
import concourse.tile as tile
from concourse import bacc
from concourse.bass import ts
from concourse.bass_utils import run_bass_kernel_spmd
from concourse.masks import make_identity

# Problem shapes (hardcoded per contract).
B, T, D = 1, 128, 4096
H, HD = 32, 128
CACHE_POS = 4096
S = CACHE_POS + T            # 4224 live cache rows
N_CORES = 8
NH = H // N_CORES            # 4 heads per core
O = NH * HD                  # 512 projection out-dims per core
NC_I = D // 128              # 32 contraction chunks for projections
NC_S = CACHE_POS // 128      # 32 old-cache s-chunks (the 33rd chunk is new k/v)
VW = HD + 4                  # v block width: v | ones | pad (4-byte aligned)
SCALE = 1.0 / float(np.sqrt(HD))
# Constant subtracted inside exp() (cancels exactly in the softmax ratio).
EXP_BIAS = -13.0
N_WARM = 72

F32 = mybir.dt.float32
BF16 = mybir.dt.bfloat16
F16 = mybir.dt.float16
F8 = mybir.dt.float8e3

WQ_DT = "f16"
WK_DT = "f16"
WV_DT = "f8"
CACHE_DT = "f8"    # k cache (test.py --cachedt override)
V_DT = "f8"        # v cache
WO_DT = "f8"       # wo weight
W_SCALE = 128.0    # host multiplies w{q,k,v}(,wo if f8) by this; x carries 1/W_SCALE
W_DT = "f16"       # legacy knob (test.py --wdt): unused

TRACE = False       # set by test.py for profiling runs
LAST_RESULT = None  # BassKernelResults of the most recent run

_NC_CACHE = {}

_MYBIR_DT = {"f32": F32, "bf16": BF16, "f16": F16, "f8": F8}
_NP_DT = {"f32": np.float32, "bf16": ml_dtypes.bfloat16,
          "f16": np.float16, "f8": ml_dtypes.float8_e3m4}


def _build_nc(wq_kind, wk_kind, wv_kind, k_kind, v_kind, wo_kind):
    """Build + compile the single-core Bass program (SPMD across 8 cores)."""
    nc = bacc.Bacc("TRN2", target_bir_lowering=False, debug=False,
                   num_devices=N_CORES, enable_asserts=False)

    dt_wq = _MYBIR_DT[wq_kind]
    dt_wk = _MYBIR_DT[wk_kind]
    dt_wv = _MYBIR_DT[wv_kind]
    dt_k = _MYBIR_DT[k_kind]
    dt_v = _MYBIR_DT[v_kind]
    dt_wo = _MYBIR_DT[wo_kind]

    xT_d = nc.dram_tensor("xT", [128, NC_I, T], F16, kind="ExternalInput").ap()
    bq_d = nc.dram_tensor("bq", [O], F16, kind="ExternalInput").ap()
    bkv_d = nc.dram_tensor("bkv", [2 * O], F16, kind="ExternalInput").ap()
    wqT_d = nc.dram_tensor("wqT", [NH, 128, NC_I, 128], dt_wq,
                           kind="ExternalInput").ap()
    wkT_d = nc.dram_tensor("wkT", [128, NC_I, O], dt_wk,
                           kind="ExternalInput").ap()
    wvT_d = nc.dram_tensor("wvT", [128, NC_I, O], dt_wv,
                           kind="ExternalInput").ap()
    # wo pieces: [piece, p, head, 1024 cols] (f8 -> 4KB lines)
    wo_d = nc.dram_tensor("wo4", [4, 128, NH, 1024], dt_wo,
                          kind="ExternalInput").ap()
    k4_d = nc.dram_tensor("k4", [2, 128, 2 * CACHE_POS], dt_k,
                          kind="ExternalInput").ap()
    v4_d = nc.dram_tensor("v4", [NH, 128, NC_S * VW], dt_v,
                          kind="ExternalInput").ap()
    y_d = nc.dram_tensor("y", [T, D], F16, kind="ExternalOutput").ap()

    with tile.TileContext(nc) as tc:
        with (
            tc.tile_pool(name="const", bufs=1) as const_pool,
            tc.tile_pool(name="wqstream", bufs=1) as wq_pool,
            tc.tile_pool(name="wkstream", bufs=1) as wk_pool,
            tc.tile_pool(name="wvstream", bufs=1) as wv_pool,
            tc.tile_pool(name="kstream", bufs=1) as k_pool,
            tc.tile_pool(name="vstream", bufs=1) as v_pool,
            tc.tile_pool(name="wostream", bufs=1) as wo_pool,
            tc.tile_pool(name="pTpool", bufs=4) as pT_pool,
            tc.tile_pool(name="small", bufs=4) as small_pool,
        ):
            # ---- constants / persistent tiles ----
            ident16 = const_pool.tile([128, 128], F16)
            make_identity(nc, ident16[:])
            ident32 = const_pool.tile([128, 128], F32)
            make_identity(nc, ident32[:])

            warm16 = const_pool.tile([128, 128], F16)
            nc.vector.memset(warm16[:], 0.01)

            expb = const_pool.tile([128, 1], F32)
            nc.vector.memset(expb[:], EXP_BIAS)

            onesrow = const_pool.tile([128, 128], F16)
            nc.vector.memset(onesrow[:], 0.0)
            nc.vector.memset(onesrow[0:1, :], 1.0)

            xT_sb = const_pool.tile([128, NC_I, T], F16)
            nc.sync.dma_start(out=xT_sb[:], in_=xT_d)

            bq_sb = const_pool.tile([128, O], F16)
            bkv_sb = const_pool.tile([128, 2 * O], F16)
            nc.vector.memset(bq_sb[:], 0.0)
            nc.vector.memset(bkv_sb[:], 0.0)
            nc.gpsimd.dma_start(out=bq_sb[0:1, :], in_=bq_d)
            nc.gpsimd.dma_start(out=bkv_sb[0:1, :], in_=bkv_d)

            qT_sb = const_pool.tile([128, NH, T], F16)
            kT_new = const_pool.tile([128, NH, T], F16)
            v_new = const_pool.tile([128, NH, VW], F16)
            aoT_sb = const_pool.tile([128, NH, T], F16)
            avO_sb = const_pool.tile([128, NH, VW], F32)
            y_sb = const_pool.tile([128, D], F16)

            nc.vector.memset(v_new[:], 0.0)
            for h in range(NH):
                nc.vector.memset(v_new[:, h, HD:HD + 1], 1.0)

            # ---- dedicated input stream tiles (no aliasing anywhere) ----
            wq_tiles = [wq_pool.tile([128, NC_I, 128], dt_wq, tag=f"wq{i}",
                                     name=f"wq{i}") for i in range(NH)]
            wk_tiles = [wk_pool.tile([128, 8, O], dt_wk, tag=f"wk{i}",
                                     name=f"wk{i}") for i in range(4)]
            wv_tiles = [wv_pool.tile([128, 16, O], dt_wv, tag=f"wv{i}",
                                     name=f"wv{i}") for i in range(2)]
            k_tiles = [k_pool.tile([128, 2 * CACHE_POS], dt_k, tag=f"k{i}",
                                   name=f"k{i}") for i in range(2)]
            v_tiles = [v_pool.tile([128, NC_S * VW], dt_v, tag=f"v{i}",
                                   name=f"v{i}") for i in range(NH)]
            wo_tiles = [wo_pool.tile([128, NH, 1024], dt_wo, tag=f"wo{i}",
                                     name=f"wo{i}") for i in range(4)]

            # ---- the stream: issue order == consumption order ----
            # x, wq0, k0, wq1, wk0, wv0, wk1, wq2, wv1, wk2, k1, wq3, wk3,
            # v0a..v3b (halves), wo0-3
            HVW = 16 * VW

            def v_halves(i):
                nc.sync.dma_start(out=v_tiles[i][:, 0:HVW],
                                  in_=v4_d[i][:, 0:HVW])
                nc.sync.dma_start(out=v_tiles[i][:, HVW:2 * HVW],
                                  in_=v4_d[i][:, HVW:2 * HVW])

            nc.sync.dma_start(out=wq_tiles[0][:], in_=wqT_d[0])
            nc.sync.dma_start(out=k_tiles[0][:], in_=k4_d[0])
            nc.sync.dma_start(out=wq_tiles[1][:], in_=wqT_d[1])
            nc.sync.dma_start(out=wk_tiles[0][:], in_=wkT_d[:, 0:8, :])
            nc.sync.dma_start(out=wv_tiles[0][:], in_=wvT_d[:, 0:16, :])
            nc.sync.dma_start(out=k_tiles[1][:], in_=k4_d[1])
            nc.sync.dma_start(out=wq_tiles[2][:], in_=wqT_d[2])
            nc.sync.dma_start(out=wq_tiles[3][:], in_=wqT_d[3])
            nc.sync.dma_start(out=wv_tiles[1][:], in_=wvT_d[:, 16:32, :])
            nc.sync.dma_start(out=wk_tiles[1][:], in_=wkT_d[:, 8:16, :])
            nc.sync.dma_start(out=wk_tiles[2][:], in_=wkT_d[:, 16:24, :])
            nc.sync.dma_start(out=wk_tiles[3][:], in_=wkT_d[:, 24:32, :])
            for i in range(4):
                v_halves(i)
            for p in range(4):
                nc.sync.dma_start(out=wo_tiles[p][:], in_=wo_d[p])

            # ---- PSUM: EARLY(qTps, kvv, kTps, av) = 4 banks (whole
            # kernel), KQ = 4 banks -> released after last scores group,
            # then Y = 4 x [128,512]. Never exceeds 8. ----
            early = tc.alloc_tile_pool(name="early", bufs=1,
                                       space="PSUM", side="left")
            kq_pool = tc.alloc_tile_pool(name="kq", bufs=2, space="PSUM",
                                         side="right")

            qT_ps = early.tile([128, 512], F32, tag="qTps", name="qTps")
            for _ in range(N_WARM):
                nc.tensor.matmul(qT_ps[:, 0:128], warm16[:], warm16[:],
                                 start=True, stop=True)
            warm_act = const_pool.tile([128, 1], F32)
            nc.scalar.activation(warm_act[:], expb[:],
                                 mybir.ActivationFunctionType.Exp)

            kT_ps = early.tile([128, 512], F32, tag="kTps", name="kTps")
            kvv_ps = early.tile([128, O], F32, tag="kvv", name="kvv")

            def qproj_head(h):
                dst = qT_ps[:, ts(h, HD)]
                for c in range(NC_I):
                    nc.tensor.matmul(dst, wq_tiles[h][:, c, :],
                                     xT_sb[:, c, :],
                                     start=(c == 0), stop=False)
                nc.tensor.matmul(dst, bq_sb[:, ts(h, HD)], onesrow[:],
                                 start=False, stop=True)
                nc.vector.tensor_copy(qT_sb[:, h, :], dst)

            def kproj_group(g):
                wkch = wk_tiles[g].rearrange("p c (h o) -> p c h o", o=HD)
                for cc in range(8):
                    c = g * 8 + cc
                    for h in range(NH):
                        nc.tensor.matmul(kT_ps[:, ts(h, HD)],
                                         wkch[:, cc, h, :],
                                         xT_sb[:, c, :],
                                         start=(c == 0 and h == 0),
                                         stop=False)
                if g == 3:
                    for h in range(NH):
                        nc.tensor.matmul(kT_ps[:, ts(h, HD)],
                                         bkv_sb[:, ts(h, HD)], onesrow[:],
                                         start=False, stop=(h == NH - 1))
                    for h in range(NH):
                        nc.vector.tensor_copy(kT_new[:, h, :],
                                              kT_ps[:, ts(h, HD)])

            def vproj_half(i):
                wvch = wv_tiles[i]
                for cc in range(16):
                    c = i * 16 + cc
                    nc.tensor.matmul(kvv_ps[:], xT_sb[:, c, :],
                                     wvch[:, cc, :],
                                     start=(c == 0), stop=False)
                if i == 1:
                    nc.tensor.matmul(kvv_ps[:], onesrow[:],
                                     bkv_sb[:, O:2 * O], start=False,
                                     stop=True)
                    for h in range(NH):
                        nc.vector.tensor_copy(v_new[:, h, 0:HD],
                                              kvv_ps[:, ts(h, HD)])

            pT_tiles = [pT_pool.tile([128, CACHE_POS], F16, tag="pT",
                                     name=f"pT{h}") for h in range(NH)]

            def scores_group(h, g):
                kT_s = k_tiles[h // 2][:, (h % 2) * CACHE_POS:
                                      (h % 2 + 1) * CACHE_POS]
                ps = kq_pool.tile([128, 1024], F32, tag="kq")
                for cc in range(8):
                    c = g * 8 + cc
                    nc.tensor.matmul(ps[:, ts(cc, 128)], kT_s[:, ts(c, 128)],
                                     qT_sb[:, h, :], start=True, stop=True)
                nc.scalar.activation(
                    pT_tiles[h][:, ts(g, 1024)], ps[:],
                    mybir.ActivationFunctionType.Exp,
                    bias=expb[:], scale=SCALE)

            av_tiles = {}

            def av_half(h, part):
                if part == 0:
                    av_tiles[h] = early.tile([128, 132], F32, tag="av",
                                             name=f"av{h}")
                av = av_tiles[h]
                v_s = v_tiles[h].rearrange("p (c o) -> p c o", o=VW)
                for cc in range(16):
                    c = part * 16 + cc
                    nc.tensor.matmul(av[:], pT_tiles[h][:, ts(c, 128)],
                                     v_s[:, c, :],
                                     start=(c == 0), stop=(c == NC_S - 1))
                if part == 1:
                    nc.vector.tensor_copy(avO_sb[:, h, :], av[:])

            # ---- phase A ----
            qproj_head(0)
            scores_group(0, 0)
            scores_group(0, 1)
            qproj_head(1)
            scores_group(0, 2)
            scores_group(0, 3)
            kproj_group(0)
            scores_group(1, 0)
            scores_group(1, 1)
            vproj_half(0)
            scores_group(1, 2)
            scores_group(1, 3)
            qproj_head(2)
            scores_group(2, 0)
            scores_group(2, 1)
            qproj_head(3)
            scores_group(2, 2)
            scores_group(2, 3)
            vproj_half(1)
            scores_group(3, 0)
            scores_group(3, 1)
            kproj_group(1)
            scores_group(3, 2)
            scores_group(3, 3)
            kproj_group(2)
            kproj_group(3)

            kq_pool.release()
            y_pool = tc.alloc_tile_pool(name="ypool", bufs=4, space="PSUM",
                                        side="right")

            # psN for all 4 heads in the freed kTps bank; ONE fused exp
            psN_all = early.tile([128, 512], F32, tag="kTps", name="psNall")
            for h in range(NH):
                nc.tensor.matmul(psN_all[:, ts(h, HD)], kT_new[:, h, :],
                                 qT_sb[:, h, :],
                                 start=(h == 0), stop=(h == NH - 1))
            pN_all = small_pool.tile([128, 512], F16, tag="pN")
            nc.scalar.activation(
                pN_all[:], psN_all[:], mybir.ActivationFunctionType.Exp,
                bias=expb[:], scale=SCALE)

            def finale_tail(h):
                avN = early.tile([128, 512], F32, tag="kvv", name=f"avN{h}")
                nc.tensor.matmul(avN[:, 0:VW], pN_all[:, ts(h, HD)],
                                 v_new[:, h, :], start=True, stop=True)
                avF = small_pool.tile([128, VW], F32, tag="avF")
                nc.vector.tensor_add(avF[:], avN[:, 0:VW], avO_sb[:, h, :])
                recip = small_pool.tile([128, 1], F32, tag="recip")
                nc.vector.reciprocal(recip[:], avF[:, HD:HD + 1])
                if wo_kind == "f8":
                    recip2 = small_pool.tile([128, 1], F32, tag="rc2")
                    nc.vector.tensor_scalar_mul(
                        recip2[:], recip[:], 1.0 / W_SCALE)
                else:
                    recip2 = recip
                ao_n = small_pool.tile([128, HD], F32, tag="ao_n")
                nc.vector.tensor_scalar_mul(ao_n[:], avF[:, 0:HD], recip2[:])
                tp2 = early.tile([128, 512], F32, tag="qTps",
                                 name=f"aotr{h}")
                nc.tensor.transpose(tp2[:, 0:128], ao_n[:], ident32[:])
                nc.vector.tensor_copy(aoT_sb[:, h, :], tp2[:, 0:128])

            av_half(0, 0)
            av_half(0, 1)
            finale_tail(0)
            av_half(1, 0)
            av_half(1, 1)
            finale_tail(1)
            av_half(2, 0)
            av_half(2, 1)
            finale_tail(2)
            av_half(3, 0)
            av_half(3, 1)
            finale_tail(3)

            # ---- output projection: 8 psum pieces of 512 cols;
            # y shipped as 4 DMAs (issue cost on gpsimd is ~0.65us each) ----
            for p8 in range(8):
                yq = y_pool.tile([128, 512], F32, tag="y", name=f"yq{p8}")
                for h in range(NH):
                    nc.tensor.matmul(
                        yq[:], aoT_sb[:, h, :],
                        wo_tiles[p8 // 2][:, h, ts(p8 % 2, 512)],
                        start=(h == 0), stop=(h == NH - 1))
                dst = y_sb[:, ts(p8, 512)]
                if p8 % 2 == 0:
                    nc.vector.tensor_copy(dst, yq[:])
                else:
                    nc.scalar.copy(dst, yq[:])
                if p8 % 2 == 1:
                    nc.gpsimd.dma_start(out=y_d[:, ts(p8 // 2, 1024)],
                                        in_=y_sb[:, ts(p8 // 2, 1024)])

            y_pool.release()
            early.release()

    nc.compile()
    return nc


def _prep_core_inputs(c, x, wq_w, wq_b, wk_w, wk_b, wv_w, wv_b, wo_w,
                      k_cache, v_cache):
    isl = slice(c * O, (c + 1) * O)
    hsl = slice(c * NH, (c + 1) * NH)
    ws = W_SCALE

    xT = np.ascontiguousarray(
        (x[0].T / ws).reshape(NC_I, 128, T).transpose(1, 0, 2),
        dtype=np.float16)

    def wT(w, dt):  # [O_slice rows] -> [128, NC_I, O] partition-major, x128
        return np.ascontiguousarray(
            (w[isl, :].T * ws).reshape(NC_I, 128, O).transpose(1, 0, 2),
            dtype=_NP_DT[dt])

    wq_base = wT(wq_w, WQ_DT)          # [128, NC_I, O]
    wqT = np.ascontiguousarray(
        wq_base.reshape(128, NC_I, NH, 128).transpose(2, 0, 1, 3))
    wkT = wT(wk_w, WK_DT)
    wvT = wT(wv_w, WV_DT)

    # wo pieces: [piece, p, head, 1024]
    wo_scale = ws if WO_DT == "f8" else 1.0
    wo3 = np.ascontiguousarray(
        (wo_w[:, isl].T * wo_scale), dtype=_NP_DT[WO_DT]).reshape(NH, 128, D)
    wo4 = np.empty((4, 128, NH, 1024), dtype=_NP_DT[WO_DT])
    for p in range(4):
        for h in range(NH):
            wo4[p, :, h, :] = wo3[h][:, p * 1024:(p + 1) * 1024]

    # k cache as head-pairs [2, 128, 2*4096]; v cache per head with a ones
    # column and pad to VW
    kT = k_cache[:CACHE_POS, hsl, :].transpose(1, 2, 0)   # [NH, 128, 4096]
    k4 = np.empty((2, 128, 2 * CACHE_POS), dtype=_NP_DT[CACHE_DT])
    for p in range(2):
        k4[p, :, 0:CACHE_POS] = kT[2 * p]
        k4[p, :, CACHE_POS:] = kT[2 * p + 1]
    v4 = np.zeros((NH, 128, NC_S, VW), dtype=_NP_DT[V_DT])
    v4[:, :, :, 0:HD] = v_cache[:CACHE_POS, hsl, :].reshape(
        NC_S, 128, NH, HD).transpose(2, 1, 0, 3)
    v4[:, :, :, HD] = 1.0

    bkv = np.empty((2 * O,), dtype=np.float16)
    bkv[0:O] = wk_b[isl]
    bkv[O:] = wv_b[isl]

    return {
        "xT": xT, "wqT": wqT, "wkT": wkT, "wvT": wvT, "wo4": wo4,
        "bq": np.ascontiguousarray(wq_b[isl], dtype=np.float16),
        "bkv": bkv,
        "k4": k4, "v4": v4.reshape(NH, 128, NC_S * VW),
    }


def kernel(x, wq_w, wq_b, wk_w, wk_b, wv_w, wv_b, wo_w, wo_b,
           k_cache, v_cache, pos, cache_pos, **_ignored):
    global LAST_RESULT
    assert int(cache_pos) == CACHE_POS, "kernel hardcodes cache_pos=4096"

    key = (WQ_DT, WK_DT, WV_DT, CACHE_DT, V_DT, WO_DT)
    if key not in _NC_CACHE:
        _NC_CACHE[key] = _build_nc(*key)
    nc = _NC_CACHE[key]

    x = np.asarray(x, dtype=np.float32)
    in_maps = [
        _prep_core_inputs(c, x, np.asarray(wq_w), np.asarray(wq_b),
                          np.asarray(wk_w), np.asarray(wk_b),
                          np.asarray(wv_w), np.asarray(wv_b),
                          np.asarray(wo_w), np.asarray(k_cache),
                          np.asarray(v_cache))
        for c in range(N_CORES)
    ]

    kwargs = {}
    if TRACE:
        _install_profile_hook()
        kwargs = {"trace": True}
    try:
        res = run_bass_kernel_spmd(nc, in_maps, list(range(N_CORES)), **kwargs)
    except Exception:
        # transient NRT failures have been observed to clear on retry
        res = run_bass_kernel_spmd(nc, in_maps, list(range(N_CORES)), **kwargs)
    LAST_RESULT = res

    y = res.results[0]["y"].astype(np.float64)
    for c in range(1, N_CORES):
        y = y + res.results[c]["y"].astype(np.float64)
    y = (y + np.asarray(wo_b, dtype=np.float64)).astype(np.float32)
    return y.reshape(B, T, D)


def _install_profile_hook():
    """Register the axon NTFF profiling hook (the agent image lacks
    antenv.axon_hooks; mirror what trn_agent_boot.trn_boot would do)."""
    import contextlib
    import ctypes
    import types

    import antenv

    if "antenv.axon_hooks" in sys.modules:
        return
    mod = types.ModuleType("antenv.axon_hooks")
    holder = {}
    mod.set_axon_ntff_profile_hook = lambda h: holder.__setitem__("h", h)
    mod.get_axon_ntff_profile_hook = lambda: holder.get("h")
    sys.modules["antenv.axon_hooks"] = mod
    antenv.axon_hooks = mod

    lib = ctypes.CDLL("/opt/axon/libaxon_pjrt.so")
    if not hasattr(lib, "axon_start_nrt_profile"):
        return
    lib.axon_start_nrt_profile.argtypes = [
        ctypes.POINTER(ctypes.c_int64), ctypes.c_size_t]
    lib.axon_start_nrt_profile.restype = ctypes.c_int64
    lib.axon_stop_nrt_profile.argtypes = [ctypes.c_char_p]
    lib.axon_stop_nrt_profile.restype = ctypes.c_int64

    @contextlib.contextmanager
    def _hook(output_dir, device_ids):
        import jax
        jax.devices()
        if device_ids:
            ids = (ctypes.c_int64 * len(device_ids))(*device_ids)
            rc = lib.axon_start_nrt_profile(ids, len(device_ids))
        else:
            rc = lib.axon_start_nrt_profile(None, 0)
        if rc != 0:
            raise RuntimeError(f"axon_start_nrt_profile rc={rc}")
        try:
            yield
        finally:
            n = lib.axon_stop_nrt_profile(str(output_dir).encode())
            if n <= 0:
                print(f"profile: rc={n} (no ntff written) in {output_dir}")

    mod.set_axon_ntff_profile_hook(_hook)


# revision 21
# speedup vs baseline: 1.0176x; 1.0176x over previous
"""Trainium2 Bass kernel for nn_AttentionWithCache (decode attention with KV cache).

Full-input contract: kernel(**inputs) takes the unsharded numpy inputs and
returns the full [1, 128, 4096] output. Internally shards tensor-parallel
over heads across 8 NeuronCores (4 heads each), runs a Bass/Tile kernel via
run_bass_kernel_spmd, and reduces the wo partial sums on gather.

Key algebraic simplification: RoPE at a single scalar position applied to
both q and the whole live cache cancels in q.k dot products; v is never
rotated — so the kernel skips RoPE entirely.

Softmax subtracts a constant 13 inside exp() (cancels in the ratio); the
denominator comes from a ones-column appended to v.

v2 redesign (from trace analysis of the 83-85us baseline):
  - EVERY streamed tile has a dedicated SBUF buffer. The baseline aliased
    stream pools (v2 reused v0's buffer etc), which made later dma_starts
    wait on compute semaphores and head-of-line-blocked the single sync
    DMA queue: k1 landed at 56us and woB at 72us, gating the whole tail.
  - Single consumption-ordered gapless stream: x, wq0, wq1, wk0, wv0, k0,
    wk1, wk2, wv1, wk3, k1, v0, v1, wq2, wq3, v2, v3, wo0..wo3. k1 lands
    ~36us so the ACT-bound exp pipeline (4 heads x 4.6us) runs with only
    one gap; wo (f8, 4 pieces) lands last feeding the output-projection
    tail directly.
  - Biases folded into the projection matmuls via a ones-row extra
    contraction chunk (kills the 0.79MB broadcast DMAs).
  - wo stored f8 e3m4 x128 (saves 2MB off the end of the stream).
  - Finales (new-row scores + normalization) run after all attention
    heads: pN exps for h0/h1 slot into the ACT gap between h1 and h2.
  - PSUM plan (8 banks, bank-granular): EARLY(qps,tr,kvk,kvv)=4 + KQ(2x2)
    =4; EARLY released after kv-proj evict, LATE(av x2, fin x2)=4; KQ
    released after last exp, Y(2x2)=4.
  - y output DMAs ride the gpsimd queue so they don't queue behind the
    input stream on sync.
"""

import sys

if "/opt/trn_rl_repo" not in sys.path:
    sys.path.insert(0, "/opt/trn_rl_repo")

import ml_dtypes
import numpy as np

import concourse.bass as bass
import concourse.mybir as mybir
import concourse.tile as tile
from concourse import bacc
from concourse.bass import ts
from concourse.bass_utils import run_bass_kernel_spmd
from concourse.masks import make_identity

# Problem shapes (hardcoded per contract).
B, T, D = 1, 128, 4096
H, HD = 32, 128
CACHE_POS = 4096
S = CACHE_POS + T            # 4224 live cache rows
N_CORES = 8
NH = H // N_CORES            # 4 heads per core
O = NH * HD                  # 512 projection out-dims per core
NC_I = D // 128              # 32 contraction chunks for projections
NC_S = CACHE_POS // 128      # 32 old-cache s-chunks (the 33rd chunk is new k/v)
VW = HD + 4                  # v block width: v | ones | pad (4-byte aligned)
SCALE = 1.0 / float(np.sqrt(HD))
# Constant subtracted inside exp() (cancels exactly in the softmax ratio).
EXP_BIAS = -13.0
N_WARM = 72

F32 = mybir.dt.float32
BF16 = mybir.dt.bfloat16
F16 = mybir.dt.float16
F8 = mybir.dt.float8e3

WQ_DT = "f16"
WK_DT = "f16"
WV_DT = "f8"
CACHE_DT = "f8"    # k cache (test.py --cachedt override)
V_DT = "f8"        # v cache
WO_DT = "f8"       # wo weight
W_SCALE = 128.0    # host multiplies w{q,k,v}(,wo if f8) by this; x carries 1/W_SCALE
W_DT = "f16"       # legacy knob (test.py --wdt): unused

TRACE = False       # set by test.py for profiling runs
LAST_RESULT = None  # BassKernelResults of the most recent run

_NC_CACHE = {}

_MYBIR_DT = {"f32": F32, "bf16": BF16, "f16": F16, "f8": F8}
_NP_DT = {"f32": np.float32, "bf16": ml_dtypes.bfloat16,
          "f16": np.float16, "f8": ml_dtypes.float8_e3m4}


def _build_nc(wq_kind, wk_kind, wv_kind, k_kind, v_kind, wo_kind):
    """Build + compile the single-core Bass program (SPMD across 8 cores)."""
    nc = bacc.Bacc("TRN2", target_bir_lowering=False, debug=False,
                   num_devices=N_CORES, enable_asserts=False)

    dt_wq = _MYBIR_DT[wq_kind]
    dt_wk = _MYBIR_DT[wk_kind]
    dt_wv = _MYBIR_DT[wv_kind]
    dt_k = _MYBIR_DT[k_kind]
    dt_v = _MYBIR_DT[v_kind]
    dt_wo = _MYBIR_DT[wo_kind]

    xT_d = nc.dram_tensor("xT", [128, NC_I, T], F16, kind="ExternalInput").ap()
    bq_d = nc.dram_tensor("bq", [O], F16, kind="ExternalInput").ap()
    bkv_d = nc.dram_tensor("bkv", [2 * O], F16, kind="ExternalInput").ap()
    wqT_d = nc.dram_tensor("wqT", [NH, 128, NC_I, 128], dt_wq,
                           kind="ExternalInput").ap()
    wkT_d = nc.dram_tensor("wkT", [128, NC_I, O], dt_wk,
                           kind="ExternalInput").ap()
    wvT_d = nc.dram_tensor("wvT", [128, NC_I, O], dt_wv,
                           kind="ExternalInput").ap()
    # wo pieces: [piece, p, head, 1024 cols] (f8 -> 4KB lines)
    wo_d = nc.dram_tensor("wo4", [4, 128, NH, 1024], dt_wo,
                          kind="ExternalInput").ap()
    k4_d = nc.dram_tensor("k4", [2, 128, 2 * CACHE_POS], dt_k,
                          kind="ExternalInput").ap()
    v4_d = nc.dram_tensor("v4", [NH, 128, NC_S * VW], dt_v,
                          kind="ExternalInput").ap()
    y_d = nc.dram_tensor("y", [T, D], F16, kind="ExternalOutput").ap()

    with tile.TileContext(nc) as tc:
        with (
            tc.tile_pool(name="const", bufs=1) as const_pool,
            tc.tile_pool(name="wqstream", bufs=1) as wq_pool,
            tc.tile_pool(name="wkstream", bufs=1) as wk_pool,
            tc.tile_pool(name="wvstream", bufs=1) as wv_pool,
            tc.tile_pool(name="kstream", bufs=1) as k_pool,
            tc.tile_pool(name="vstream", bufs=1) as v_pool,
            tc.tile_pool(name="wostream", bufs=1) as wo_pool,
            tc.tile_pool(name="pTpool", bufs=4) as pT_pool,
            tc.tile_pool(name="small", bufs=4) as small_pool,
        ):
            # ---- constants / persistent tiles ----
            ident16 = const_pool.tile([128, 128], F16)
            make_identity(nc, ident16[:])
            ident32 = const_pool.tile([128, 128], F32)
            make_identity(nc, ident32[:])

            warm16 = const_pool.tile([128, 128], F16)
            nc.vector.memset(warm16[:], 0.01)

            expb = const_pool.tile([128, 1], F32)
            nc.vector.memset(expb[:], EXP_BIAS)

            onesrow = const_pool.tile([128, 128], F16)
            nc.vector.memset(onesrow[:], 0.0)
            nc.vector.memset(onesrow[0:1, :], 1.0)

            xT_sb = const_pool.tile([128, NC_I, T], F16)
            nc.sync.dma_start(out=xT_sb[:], in_=xT_d)

            bq_sb = const_pool.tile([128, O], F16)
            bkv_sb = const_pool.tile([128, 2 * O], F16)
            nc.vector.memset(bq_sb[:], 0.0)
            nc.vector.memset(bkv_sb[:], 0.0)
            nc.gpsimd.dma_start(out=bq_sb[0:1, :], in_=bq_d)
            nc.gpsimd.dma_start(out=bkv_sb[0:1, :], in_=bkv_d)

            qT_sb = const_pool.tile([128, NH, T], F16)
            kT_new = const_pool.tile([128, NH, T], F16)
            v_new = const_pool.tile([128, NH, VW], F16)
            aoT_sb = const_pool.tile([128, NH, T], F16)
            y_sb = const_pool.tile([128, D], F16)

            nc.vector.memset(v_new[:], 0.0)
            for h in range(NH):
                nc.vector.memset(v_new[:, h, HD:HD + 1], 1.0)

            # ---- dedicated input stream tiles (no aliasing anywhere) ----
            wq_tiles = [wq_pool.tile([128, NC_I, 128], dt_wq, tag=f"wq{i}",
                                     name=f"wq{i}") for i in range(NH)]
            wk_tiles = [wk_pool.tile([128, 8, O], dt_wk, tag=f"wk{i}",
                                     name=f"wk{i}") for i in range(4)]
            wv_tiles = [wv_pool.tile([128, 16, O], dt_wv, tag=f"wv{i}",
                                     name=f"wv{i}") for i in range(2)]
            k_tiles = [k_pool.tile([128, 2 * CACHE_POS], dt_k, tag=f"k{i}",
                                   name=f"k{i}") for i in range(2)]
            v_tiles = [v_pool.tile([128, NC_S * VW], dt_v, tag=f"v{i}",
                                   name=f"v{i}") for i in range(NH)]
            wo_tiles = [wo_pool.tile([128, NH, 1024], dt_wo, tag=f"wo{i}",
                                     name=f"wo{i}") for i in range(4)]

            # ---- the stream: issue order == consumption order ----
            # x, wq0, k0, wq1, wk0, wv0, wk1, wq2, wv1, wk2, k1, wq3, wk3,
            # v0a..v3b (halves), wo0-3
            HVW = 16 * VW

            def v_halves(i):
                nc.sync.dma_start(out=v_tiles[i][:, 0:HVW],
                                  in_=v4_d[i][:, 0:HVW])
                nc.sync.dma_start(out=v_tiles[i][:, HVW:2 * HVW],
                                  in_=v4_d[i][:, HVW:2 * HVW])

            nc.sync.dma_start(out=wq_tiles[0][:], in_=wqT_d[0])
            nc.sync.dma_start(out=k_tiles[0][:], in_=k4_d[0])
            nc.sync.dma_start(out=wq_tiles[1][:], in_=wqT_d[1])
            nc.sync.dma_start(out=wk_tiles[0][:], in_=wkT_d[:, 0:8, :])
            nc.sync.dma_start(out=wv_tiles[0][:], in_=wvT_d[:, 0:16, :])
            nc.sync.dma_start(out=k_tiles[1][:], in_=k4_d[1])
            nc.sync.dma_start(out=wq_tiles[2][:], in_=wqT_d[2])
            nc.sync.dma_start(out=wq_tiles[3][:], in_=wqT_d[3])
            nc.sync.dma_start(out=wv_tiles[1][:], in_=wvT_d[:, 16:32, :])
            nc.sync.dma_start(out=wk_tiles[1][:], in_=wkT_d[:, 8:16, :])
            nc.sync.dma_start(out=wk_tiles[2][:], in_=wkT_d[:, 16:24, :])
            nc.sync.dma_start(out=wk_tiles[3][:], in_=wkT_d[:, 24:32, :])
            for i in range(4):
                v_halves(i)
            for p in range(4):
                nc.sync.dma_start(out=wo_tiles[p][:], in_=wo_d[p])

            # ---- PSUM: EARLY(qTps, kvv, kTps, av) = 4 banks (whole
            # kernel), KQ = 4 banks -> released after last scores group,
            # then Y = 4 x [128,512]. Never exceeds 8. ----
            early = tc.alloc_tile_pool(name="early", bufs=1,
                                       space="PSUM", side="left")
            kq_pool = tc.alloc_tile_pool(name="kq", bufs=2, space="PSUM",
                                         side="right")

            qT_ps = early.tile([128, 512], F32, tag="qTps", name="qTps")
            for _ in range(N_WARM):
                nc.tensor.matmul(qT_ps[:, 0:128], warm16[:], warm16[:],
                                 start=True, stop=True)
            warm_act = const_pool.tile([128, 1], F32)
            nc.scalar.activation(warm_act[:], expb[:],
                                 mybir.ActivationFunctionType.Exp)

            kT_ps = early.tile([128, 512], F32, tag="kTps", name="kTps")
            kvv_ps = early.tile([128, O], F32, tag="kvv", name="kvv")

            def qproj_head(h):
                dst = qT_ps[:, ts(h, HD)]
                for c in range(NC_I):
                    nc.tensor.matmul(dst, wq_tiles[h][:, c, :],
                                     xT_sb[:, c, :],
                                     start=(c == 0), stop=False)
                nc.tensor.matmul(dst, bq_sb[:, ts(h, HD)], onesrow[:],
                                 start=False, stop=True)
                nc.vector.tensor_copy(qT_sb[:, h, :], dst)

            def kproj_group(g):
                wkch = wk_tiles[g].rearrange("p c (h o) -> p c h o", o=HD)
                for cc in range(8):
                    c = g * 8 + cc
                    for h in range(NH):
                        nc.tensor.matmul(kT_ps[:, ts(h, HD)],
                                         wkch[:, cc, h, :],
                                         xT_sb[:, c, :],
                                         start=(c == 0 and h == 0),
                                         stop=False)
                if g == 3:
                    for h in range(NH):
                        nc.tensor.matmul(kT_ps[:, ts(h, HD)],
                                         bkv_sb[:, ts(h, HD)], onesrow[:],
                                         start=False, stop=(h == NH - 1))
                    for h in range(NH):
                        nc.vector.tensor_copy(kT_new[:, h, :],
                                              kT_ps[:, ts(h, HD)])

            def vproj_half(i):
                wvch = wv_tiles[i]
                for cc in range(16):
                    c = i * 16 + cc
                    nc.tensor.matmul(kvv_ps[:], xT_sb[:, c, :],
                                     wvch[:, cc, :],
                                     start=(c == 0), stop=False)
                if i == 1:
                    nc.tensor.matmul(kvv_ps[:], onesrow[:],
                                     bkv_sb[:, O:2 * O], start=False,
                                     stop=True)
                    for h in range(NH):
                        nc.vector.tensor_copy(v_new[:, h, 0:HD],
                                              kvv_ps[:, ts(h, HD)])

            pT_tiles = [pT_pool.tile([128, CACHE_POS], F16, tag="pT",
                                     name=f"pT{h}") for h in range(NH)]

            def scores_group(h, g):
                kT_s = k_tiles[h // 2][:, (h % 2) * CACHE_POS:
                                      (h % 2 + 1) * CACHE_POS]
                ps = kq_pool.tile([128, 1024], F32, tag="kq")
                for cc in range(8):
                    c = g * 8 + cc
                    nc.tensor.matmul(ps[:, ts(cc, 128)], kT_s[:, ts(c, 128)],
                                     qT_sb[:, h, :], start=True, stop=True)
                nc.scalar.activation(
                    pT_tiles[h][:, ts(g, 1024)], ps[:],
                    mybir.ActivationFunctionType.Exp,
                    bias=expb[:], scale=SCALE)

            def av_finale(h, pN_all):
                # old-cache av accumulation; the new-row contribution joins
                # the SAME psum group (start=False), so no avO copy/add.
                # h0/h2 use the "av" bank, h1/h3 the retired "kTps" bank.
                if h % 2 == 0:
                    av = early.tile([128, 132], F32, tag="av",
                                    name=f"av{h}")[:, 0:VW]
                else:
                    av = early.tile([128, 512], F32, tag="kTps",
                                    name=f"av{h}")[:, 0:VW]
                v_s = v_tiles[h].rearrange("p (c o) -> p c o", o=VW)
                for c in range(NC_S):
                    nc.tensor.matmul(av, pT_tiles[h][:, ts(c, 128)],
                                     v_s[:, c, :],
                                     start=(c == 0), stop=False)
                nc.tensor.matmul(av, pN_all[:, ts(h, HD)], v_new[:, h, :],
                                 start=False, stop=True)
                recip = small_pool.tile([128, 1], F32, tag="recip")
                nc.vector.reciprocal(recip[:], av[:, HD:HD + 1])
                ao_n = small_pool.tile([128, HD], F32, tag="ao_n")
                if wo_kind == "f8":
                    # one fused DVE op: av * recip * (1/W_SCALE)
                    nc.vector.tensor_scalar(
                        ao_n[:], av[:, 0:HD], recip[:], 1.0 / W_SCALE,
                        mybir.AluOpType.mult, mybir.AluOpType.mult)
                else:
                    nc.vector.tensor_scalar_mul(ao_n[:], av[:, 0:HD],
                                                recip[:])
                tp2 = early.tile([128, 512], F32, tag="qTps",
                                 name=f"aotr{h}")
                nc.tensor.transpose(tp2[:, 0:128], ao_n[:], ident32[:])
                nc.vector.tensor_copy(aoT_sb[:, h, :], tp2[:, 0:128])

            # ---- phase A ----
            qproj_head(0)
            scores_group(0, 0)
            scores_group(0, 1)
            qproj_head(1)
            scores_group(0, 2)
            scores_group(0, 3)
            kproj_group(0)
            scores_group(1, 0)
            scores_group(1, 1)
            vproj_half(0)
            scores_group(1, 2)
            scores_group(1, 3)
            qproj_head(2)
            scores_group(2, 0)
            scores_group(2, 1)
            qproj_head(3)
            scores_group(2, 2)
            scores_group(2, 3)
            vproj_half(1)
            scores_group(3, 0)
            scores_group(3, 1)
            kproj_group(1)
            scores_group(3, 2)
            scores_group(3, 3)
            kproj_group(2)
            kproj_group(3)

            kq_pool.release()
            y_pool = tc.alloc_tile_pool(name="ypool", bufs=4, space="PSUM",
                                        side="right")

            # psN for all 4 heads in the freed kTps bank; ONE fused exp
            psN_all = early.tile([128, 512], F32, tag="kTps", name="psNall")
            for h in range(NH):
                nc.tensor.matmul(psN_all[:, ts(h, HD)], kT_new[:, h, :],
                                 qT_sb[:, h, :],
                                 start=(h == 0), stop=(h == NH - 1))
            pN_all = small_pool.tile([128, 512], F16, tag="pN")
            nc.scalar.activation(
                pN_all[:], psN_all[:], mybir.ActivationFunctionType.Exp,
                bias=expb[:], scale=SCALE)

            av_finale(0, pN_all)
            av_finale(1, pN_all)
            av_finale(2, pN_all)

            # wo pieces 0-1: heads 0-2 accumulate while av h3 still runs
            yqs = [y_pool.tile([128, 512], F32, tag="y", name=f"yq{p8}")
                   for p8 in range(8)]
            for p8 in range(2):
                for h in range(3):
                    nc.tensor.matmul(
                        yqs[p8][:], aoT_sb[:, h, :],
                        wo_tiles[p8 // 2][:, h, ts(p8 % 2, 512)],
                        start=(h == 0), stop=False)

            av_finale(3, pN_all)

            def wo_finish(p8, heads):
                for h in heads:
                    nc.tensor.matmul(
                        yqs[p8][:], aoT_sb[:, h, :],
                        wo_tiles[p8 // 2][:, h, ts(p8 % 2, 512)],
                        start=(h == 0), stop=(h == NH - 1))
                dst = y_sb[:, ts(p8, 512)]
                if p8 % 2 == 0:
                    nc.vector.tensor_copy(dst, yqs[p8][:])
                else:
                    nc.scalar.copy(dst, yqs[p8][:])
                if p8 % 2 == 1:
                    nc.gpsimd.dma_start(out=y_d[:, ts(p8 // 2, 1024)],
                                        in_=y_sb[:, ts(p8 // 2, 1024)])

            wo_finish(0, [3])
            wo_finish(1, [3])
            for p8 in range(2, 8):
                wo_finish(p8, list(range(NH)))

            y_pool.release()
            early.release()

    nc.compile()
    return nc


def _prep_core_inputs(c, x, wq_w, wq_b, wk_w, wk_b, wv_w, wv_b, wo_w,
                      k_cache, v_cache):
    isl = slice(c * O, (c + 1) * O)
    hsl = slice(c * NH, (c + 1) * NH)
    ws = W_SCALE

    xT = np.ascontiguousarray(
        (x[0].T / ws).reshape(NC_I, 128, T).transpose(1, 0, 2),
        dtype=np.float16)

    def wT(w, dt):  # [O_slice rows] -> [128, NC_I, O] partition-major, x128
        return np.ascontiguousarray(
            (w[isl, :].T * ws).reshape(NC_I, 128, O).transpose(1, 0, 2),
            dtype=_NP_DT[dt])

    wq_base = wT(wq_w, WQ_DT)          # [128, NC_I, O]
    wqT = np.ascontiguousarray(
        wq_base.reshape(128, NC_I, NH, 128).transpose(2, 0, 1, 3))
    wkT = wT(wk_w, WK_DT)
    wvT = wT(wv_w, WV_DT)

    # wo pieces: [piece, p, head, 1024]
    wo_scale = ws if WO_DT == "f8" else 1.0
    wo3 = np.ascontiguousarray(
        (wo_w[:, isl].T * wo_scale), dtype=_NP_DT[WO_DT]).reshape(NH, 128, D)
    wo4 = np.empty((4, 128, NH, 1024), dtype=_NP_DT[WO_DT])
    for p in range(4):
        for h in range(NH):
            wo4[p, :, h, :] = wo3[h][:, p * 1024:(p + 1) * 1024]

    # k cache as head-pairs [2, 128, 2*4096]; v cache per head with a ones
    # column and pad to VW
    kT = k_cache[:CACHE_POS, hsl, :].transpose(1, 2, 0)   # [NH, 128, 4096]
    k4 = np.empty((2, 128, 2 * CACHE_POS), dtype=_NP_DT[CACHE_DT])
    for p in range(2):
        k4[p, :, 0:CACHE_POS] = kT[2 * p]
        k4[p, :, CACHE_POS:] = kT[2 * p + 1]
    v4 = np.zeros((NH, 128, NC_S, VW), dtype=_NP_DT[V_DT])
    v4[:, :, :, 0:HD] = v_cache[:CACHE_POS, hsl, :].reshape(
        NC_S, 128, NH, HD).transpose(2, 1, 0, 3)
    v4[:, :, :, HD] = 1.0

    bkv = np.empty((2 * O,), dtype=np.float16)
    bkv[0:O] = wk_b[isl]
    bkv[O:] = wv_b[isl]

    return {
        "xT": xT, "wqT": wqT, "wkT": wkT, "wvT": wvT, "wo4": wo4,
        "bq": np.ascontiguousarray(wq_b[isl], dtype=np.float16),
        "bkv": bkv,
        "k4": k4, "v4": v4.reshape(NH, 128, NC_S * VW),
    }


def kernel(x, wq_w, wq_b, wk_w, wk_b, wv_w, wv_b, wo_w, wo_b,
           k_cache, v_cache, pos, cache_pos, **_ignored):
    global LAST_RESULT
    assert int(cache_pos) == CACHE_POS, "kernel hardcodes cache_pos=4096"

    key = (WQ_DT, WK_DT, WV_DT, CACHE_DT, V_DT, WO_DT)
    if key not in _NC_CACHE:
        _NC_CACHE[key] = _build_nc(*key)
    nc = _NC_CACHE[key]

    x = np.asarray(x, dtype=np.float32)
    in_maps = [
        _prep_core_inputs(c, x, np.asarray(wq_w), np.asarray(wq_b),
                          np.asarray(wk_w), np.asarray(wk_b),
                          np.asarray(wv_w), np.asarray(wv_b),
                          np.asarray(wo_w), np.asarray(k_cache),
                          np.asarray(v_cache))
        for c in range(N_CORES)
    ]

    kwargs = {}
    if TRACE:
        _install_profile_hook()
        kwargs = {"trace": True}
    try:
        res = run_bass_kernel_spmd(nc, in_maps, list(range(N_CORES)), **kwargs)
    except Exception:
        # transient NRT failures have been observed to clear on retry
        res = run_bass_kernel_spmd(nc, in_maps, list(range(N_CORES)), **kwargs)
    LAST_RESULT = res

    y = res.results[0]["y"].astype(np.float64)
    for c in range(1, N_CORES):
        y = y + res.results[c]["y"].astype(np.float64)
    y = (y + np.asarray(wo_b, dtype=np.float64)).astype(np.float32)
    return y.reshape(B, T, D)


def _install_profile_hook():
    """Register the axon NTFF profiling hook (the agent image lacks
    antenv.axon_hooks; mirror what trn_agent_boot.trn_boot would do)."""
    import contextlib
    import ctypes
    import types

    import antenv

    if "antenv.axon_hooks" in sys.modules:
        return
    mod = types.ModuleType("antenv.axon_hooks")
    holder = {}
    mod.set_axon_ntff_profile_hook = lambda h: holder.__setitem__("h", h)
    mod.get_axon_ntff_profile_hook = lambda: holder.get("h")
    sys.modules["antenv.axon_hooks"] = mod
    antenv.axon_hooks = mod

    lib = ctypes.CDLL("/opt/axon/libaxon_pjrt.so")
    if not hasattr(lib, "axon_start_nrt_profile"):
        return
    lib.axon_start_nrt_profile.argtypes = [
        ctypes.POINTER(ctypes.c_int64), ctypes.c_size_t]
    lib.axon_start_nrt_profile.restype = ctypes.c_int64
    lib.axon_stop_nrt_profile.argtypes = [ctypes.c_char_p]
    lib.axon_stop_nrt_profile.restype = ctypes.c_int64

    @contextlib.contextmanager
    def _hook(output_dir, device_ids):
        import jax
        jax.devices()
        if device_ids:
            ids = (ctypes.c_int64 * len(device_ids))(*device_ids)
            rc = lib.axon_start_nrt_profile(ids, len(device_ids))
        else:
            rc = lib.axon_start_nrt_profile(None, 0)
        if rc != 0:
            raise RuntimeError(f"axon_start_nrt_profile rc={rc}")
        try:
            yield
        finally:
            n = lib.axon_stop_nrt_profile(str(output_dir).encode())
            if n <= 0:
                print(f"profile: rc={n} (no ntff written) in {output_dir}")

    mod.set_axon_ntff_profile_hook(_hook)
